# revision 19
# baseline (speedup 1.0000x reference)
"""DeepSeekV3-style MoE layer on 8 Trainium2 NeuronCores.

Strategy (expert-parallel, host-side dispatch):
  - Host computes the sigmoid gate + top-2 routing (tiny: [8192,2048]@[2048,16]),
    gathers each expert's tokens. Experts are paired largest-with-smallest and
    sharded 2-per-core; per-slot capacities C0/C1 are the max count over the
    slot's 8 experts (exact, no rounding). The shared expert is data-parallel
    (1024 tokens per core).
  - Each core runs the same Bass/Tile program: 3 SwiGLU "units"
    (shared + 2 experts), weight-stationary matmuls at N<=512, f32r
    (full-rate reduced-precision fp32) on the PE, fp32 PSUM accumulation.
  - Gating scale is applied on-device during PSUM->SBUF evacuation; host
    scatter-adds expert outputs back and adds the shared output.

Layouts (host-prepared so every DMA is wide & contiguous):
  x*T   [16,128,n]        tokens transposed, h-tile major
  w1p   [3,11,128,2048]   phase-1 lhsT packs: [u][it][p=h%128][ht*128+j(=i%128)]
  w3p   same
  w2t   [3,11,128,2048]   w2 transposed: [u][it][p=i%128][h]
  gs*   [ct,128,1]        per-token gating scale for each local expert slot
Outputs: ys [1024,2048], ye0 [C0,2048], ye1 [C1,2048] (token-major, fp32).
"""

import os
import sys

import numpy as np

if "/opt/trn_rl_repo" not in sys.path:
    sys.path.insert(0, "/opt/trn_rl_repo")

import concourse.bass as bass
import concourse.bacc as bacc
import concourse.mybir as mybir
import concourse.tile as tile
from concourse.bass_utils import run_bass_kernel_spmd

B, S, H, I, E, TOPK = 4, 2048, 2048, 1408, 16, 2
T = B * S               # 8192 tokens
NCORES = 8
NS = T // NCORES        # shared-expert tokens per core
HT, IT = H // 128, I // 128   # 16, 11
EPC = E // NCORES       # experts per core = 2

MM_MODE = os.environ.get("MOE_MM_MODE", "fp16")   # "fp16" | "f32r" | "bf16" | "f32"

LAST_RESULTS = None     # BassKernelResults of the last run (for test harness)

_PROGRAM_CACHE = {}
_PACK_CACHE = {}


def _sigmoid(x):
    out = np.empty_like(x)
    np.negative(x, out=out)
    np.exp(out, out=out)
    out += 1.0
    np.reciprocal(out, out=out)
    return out


def _chunks(n):
    """Split n into chunks <=512, all >=256 when n allows (f32r matmul runs
    at 1/4 rate below a 256-wide moving dim)."""
    out, rem = [], n
    while rem > 0:
        if rem <= 512:
            c = rem
        elif rem >= 768:
            c = 512
        else:  # rem in (512, 768): split so both pieces are >= 256
            c = rem - 256
        out.append(c)
        rem -= c
    return out


def _build_program(caps, mode):
    """caps = (C0, C1): exact token capacity of the two local expert slots."""
    key = (caps, mode)
    if key in _PROGRAM_CACHE:
        return _PROGRAM_CACHE[key]

    if mode == "bf16":
        in_dt = mybir.dt.bfloat16
    elif mode == "fp16":
        in_dt = mybir.dt.float16
    elif mode == "f32r":
        in_dt = mybir.dt.float32r
    else:
        in_dt = mybir.dt.float32
    f32 = mybir.dt.float32
    n_units = [NS, caps[0], caps[1]]
    cts = [-(-n // 128) for n in n_units]
    CW = max(n_units)       # tile width shared by xt/g tags

    nc = bacc.Bacc("TRN2", target_bir_lowering=False, debug=False)

    xT = [nc.dram_tensor(f"x{u}T", [HT, 128, n_units[u]], in_dt,
                         kind="ExternalInput").ap() for u in range(3)]
    w1p = nc.dram_tensor("w1p", [3, IT, 128, H], in_dt, kind="ExternalInput").ap()
    w3p = nc.dram_tensor("w3p", [3, IT, 128, H], in_dt, kind="ExternalInput").ap()
    w2t = nc.dram_tensor("w2t", [3, IT, 128, H], in_dt, kind="ExternalInput").ap()
    gs = [None] + [nc.dram_tensor(f"gs{u}", [128, cts[u]], f32,
                                  kind="ExternalInput").ap() for u in (1, 2)]
    yo = [nc.dram_tensor(["ys", "ye0", "ye1"][u], [n_units[u], H], f32,
                         kind="ExternalOutput").ap() for u in range(3)]

    # DMA-queue plan (one HW queue per engine, FIFO): sync carries only the
    # activation loads, scalar only the output writes (its sigmoid compute is
    # phase-1-only), gpsimd all weight streams. Emission order = descriptor
    # order, so prefetches are hoisted ahead of the compute that needs them.
    with tile.TileContext(nc) as tc:
        with (
            tc.tile_pool(name="xt", bufs=HT) as xt_pool,
            tc.tile_pool(name="g", bufs=IT) as g_pool,
            tc.tile_pool(name="w13", bufs=6) as w13_pool,
            tc.tile_pool(name="w2", bufs=2 * IT) as w2_pool,
            tc.tile_pool(name="gsb", bufs=2) as gs_pool,
            tc.tile_pool(name="ot", bufs=4) as out_pool,
            tc.tile_pool(name="ps1", bufs=4, space="PSUM") as ps1_pool,
            tc.tile_pool(name="ps2", bufs=4, space="PSUM") as ps2_pool,
        ):
            def load_xt(u):
                n_u = n_units[u]
                xts = [xt_pool.tile([128, CW], in_dt, tag="xt",
                                    name=f"xt{u}_{ht}") for ht in range(HT)]
                if u == 0:
                    # racing the kernel start: two queues, and land the first
                    # 512 token-columns of every h-tile first so the first
                    # phase-1 chunk group can begin while the rest streams
                    w0 = min(512, n_u)
                    for ht in range(HT):
                        eng = nc.scalar if ht % 2 else nc.sync
                        eng.dma_start(out=xts[ht][:, :w0], in_=xT[u][ht][:, :w0])
                    if w0 < n_u:
                        for ht in range(HT):
                            eng = nc.scalar if ht % 2 else nc.sync
                            eng.dma_start(out=xts[ht][:, w0:n_u],
                                          in_=xT[u][ht][:, w0:])
                else:
                    for ht in range(HT):
                        nc.sync.dma_start(out=xts[ht][:, :n_u], in_=xT[u][ht])
                return xts

            # PE warm-up: ~160 dummy matmuls fill the otherwise-idle startup
            # DMA window so the HAM clock gate reaches 8/8 (2.4 GHz) before
            # the first real matmul; the scratch PSUM is never read.
            wrm = out_pool.tile([128, 128], in_dt, tag="ot", name="wrm")
            nc.vector.memset(wrm[:], 1.0)
            psd = ps2_pool.tile([128, 512], f32, tag="ps2", name="psd")
            for k in range(160):
                nc.tensor.matmul(psd[:, :128], wrm[:], wrm[:],
                                 start=True, stop=True)

            xts = load_xt(0)
            for u in range(3):
                n_u = n_units[u]

                # ---- weight-stream emission (gpsimd): w13 it0/it1 first,
                # then this unit's gating vector + first w2 group, then the
                # rest of w13 (slot-waits pace the queue to compute progress)
                w13s = []
                for it in range(IT):
                    w1t = w13_pool.tile([128, H], in_dt, tag="w13",
                                        name=f"w1t{u}_{it}")
                    w3t = w13_pool.tile([128, H], in_dt, tag="w13",
                                        name=f"w3t{u}_{it}")
                    w13s.append((w1t, w3t))
                    nc.gpsimd.dma_start(out=w1t[:], in_=w1p[u, it])
                    nc.gpsimd.dma_start(out=w3t[:], in_=w3p[u, it])
                    if it == 1:
                        if u > 0:
                            gst = gs_pool.tile([128, cts[u]], f32, tag="gsb",
                                               name=f"gst{u}")
                            nc.gpsimd.dma_start(out=gst[:], in_=gs[u])
                        w2s = []
                        for w2i in range(IT):
                            w2tile = w2_pool.tile([128, 512], in_dt, tag="w2",
                                                  name=f"w2_{u}_0_{w2i}")
                            nc.gpsimd.dma_start(
                                out=w2tile[:], in_=w2t[u, w2i, :, 0:512])
                            w2s.append(w2tile)

                # ---- phase 1: G^T[i, t] = silu(W1 xT) * (W3 xT) ----
                gts = []
                for it in range(IT):
                    w1t, w3t = w13s[it]
                    gt = g_pool.tile([128, CW], in_dt, tag="g", name=f"g{u}_{it}")
                    gts.append(gt)
                    c0 = 0
                    for w in _chunks(n_u):
                        ps1 = ps1_pool.tile([128, 512], f32, tag="ps1",
                                            name=f"ps1_{u}_{it}_{c0}")
                        ps3 = ps1_pool.tile([128, 512], f32, tag="ps1",
                                            name=f"ps3_{u}_{it}_{c0}")
                        for ht in range(HT):
                            nc.tensor.matmul(
                                ps1[:, :w], w1t[:, ht * 128:(ht + 1) * 128],
                                xts[ht][:, c0:c0 + w],
                                start=(ht == 0), stop=(ht == HT - 1))
                        for ht in range(HT):
                            nc.tensor.matmul(
                                ps3[:, :w], w3t[:, ht * 128:(ht + 1) * 128],
                                xts[ht][:, c0:c0 + w],
                                start=(ht == 0), stop=(ht == HT - 1))
                        # silu(h1)*h3 = sigmoid(h1)*h1*h3 (Silu not in CoreSim)
                        gsl = gt[:, c0:c0 + w]
                        nc.scalar.activation(gsl, ps1[:, :w],
                                             mybir.ActivationFunctionType.Sigmoid)
                        nc.vector.tensor_mul(gsl, gsl, ps1[:, :w])
                        nc.vector.tensor_mul(gsl, gsl, ps3[:, :w])
                        c0 += w

                # next unit's activations stream during phase 2
                if u < 2:
                    next_xts = load_xt(u + 1)

                # ---- phase 2: Y[t, h] = G^T.T @ W2^T, +gating scale ----
                for hc in range(H // 512):
                    if hc + 1 < H // 512:   # prefetch next w2 group
                        nxt = []
                        for w2i in range(IT):
                            w2tile = w2_pool.tile([128, 512], in_dt, tag="w2",
                                                  name=f"w2_{u}_{hc + 1}_{w2i}")
                            nc.gpsimd.dma_start(
                                out=w2tile[:],
                                in_=w2t[u, w2i, :,
                                        (hc + 1) * 512:(hc + 2) * 512])
                            nxt.append(w2tile)
                    for tt in range(cts[u]):
                        m = min(128, n_u - tt * 128)
                        ps = ps2_pool.tile([128, 512], f32, tag="ps2",
                                           name=f"ps2_{u}_{hc}_{tt}")
                        for it in range(IT):
                            nc.tensor.matmul(
                                ps[:m], gts[it][:, tt * 128:tt * 128 + m],
                                w2s[it][:],
                                start=(it == 0), stop=(it == IT - 1))
                        ot = out_pool.tile([128, 512], f32, tag="ot",
                                           name=f"ot{u}_{hc}_{tt}")
                        if u == 0:
                            nc.vector.tensor_copy(ot[:m], ps[:m])
                        else:
                            nc.vector.tensor_scalar(
                                ot[:m], ps[:m], gst[:m, tt:tt + 1], None,
                                mybir.AluOpType.mult)
                        nc.scalar.dma_start(
                            out=yo[u][tt * 128:tt * 128 + m,
                                      hc * 512:(hc + 1) * 512],
                            in_=ot[:m])
                    if hc + 1 < H // 512:
                        w2s = nxt
                if u < 2:
                    xts = next_xts

    nc.compile()
    _PROGRAM_CACHE[key] = nc
    return nc


def _np_dt(mode):
    if mode == "bf16":
        import ml_dtypes
        return np.dtype(ml_dtypes.bfloat16)
    if mode == "fp16":
        return np.dtype(np.float16)
    return np.dtype(np.float32)


def _pack_w13(w, dt):
    """[I,H] -> [IT,128,H] with [it, p, ht*128+j] = w[it*128+j, ht*128+p]."""
    a = np.ascontiguousarray(
        w.reshape(IT, 128, HT, 128).transpose(0, 3, 2, 1), dtype=dt)
    return a.reshape(IT, 128, H)


def _pack_w2(w, dt):
    """[H,I] -> [IT,128,H]  (= w.T tiled along I)."""
    return np.ascontiguousarray(w.T.reshape(IT, 128, H), dtype=dt)


def _pack_all_weights(shared_w1, shared_w3, shared_w2, w1, w3, w2, mode):
    key = (id(w1), id(w2), id(w3), mode)
    if _PACK_CACHE.get("key") == key:
        return _PACK_CACHE["val"]
    dt = _np_dt(mode)
    p1 = [_pack_w13(shared_w1, dt)] + [_pack_w13(w1[e], dt) for e in range(E)]
    p3 = [_pack_w13(shared_w3, dt)] + [_pack_w13(w3[e], dt) for e in range(E)]
    p2 = [_pack_w2(shared_w2, dt)] + [_pack_w2(w2[e], dt) for e in range(E)]
    val = (p1, p3, p2)
    _PACK_CACHE["key"] = key
    _PACK_CACHE["val"] = val
    return val


def _prepare(hidden_states, gate_w, bias, shared_w1, shared_w3, shared_w2,
             w1, w3, w2, mode):
    """Host routing + per-core input maps. Returns (nc, in_maps, meta)."""
    x = np.ascontiguousarray(hidden_states.reshape(T, H), dtype=np.float32)

    scores = _sigmoid(x @ gate_w.T.astype(np.float32))
    routing = scores + bias.astype(np.float32)[None, :]
    topk = np.argsort(-routing, axis=1, kind="stable")[:, :TOPK]
    sel = np.take_along_axis(scores, topk, axis=1)
    gating = (sel / sel.sum(axis=1, keepdims=True)).astype(np.float32)

    flat_t = np.repeat(np.arange(T), TOPK)
    flat_e = topk.ravel()
    flat_g = gating.ravel()
    order = np.argsort(flat_e, kind="stable")
    flat_t, flat_g = flat_t[order], flat_g[order]
    counts = np.bincount(flat_e, minlength=E)
    offs = np.zeros(E + 1, np.int64)
    np.cumsum(counts, out=offs[1:])

    # pair largest with smallest: slot0 = rank c, slot1 = rank 15-c
    rank = np.argsort(-counts, kind="stable")
    slot_experts = [(int(rank[c]), int(rank[E - 1 - c])) for c in range(NCORES)]
    C0 = max(1, int(counts[rank[0]]))
    C1 = max(1, int(counts[rank[NCORES]]))
    caps = (C0, C1)

    nc = _build_program(caps, mode)
    dt = _np_dt(mode)

    p1, p3, p2 = _pack_all_weights(shared_w1, shared_w3, shared_w2,
                                   w1, w3, w2, mode)
    xc = x.astype(dt, copy=False)

    tok_ids = []
    in_maps = []
    for c in range(NCORES):
        im = {"x0T": np.ascontiguousarray(
            xc[c * NS:(c + 1) * NS].T).reshape(HT, 128, NS)}
        ids_pair = []
        for j, e in enumerate(slot_experts[c]):
            Cj = caps[j]
            ids = flat_t[offs[e]:offs[e + 1]]
            ids_pair.append(ids)
            n = len(ids)
            xg = np.zeros((Cj, H), dt)
            xg[:n] = xc[ids]
            im[f"x{j + 1}T"] = np.ascontiguousarray(xg.T).reshape(HT, 128, Cj)
            ct = -(-Cj // 128)
            gsc = np.zeros((ct, 128), np.float32)
            gsc.reshape(-1)[:n] = flat_g[offs[e]:offs[e + 1]]
            im[f"gs{j + 1}"] = np.ascontiguousarray(gsc.T)   # [128, ct]
        e0, e1 = slot_experts[c]
        im["w1p"] = np.stack([p1[0], p1[1 + e0], p1[1 + e1]])
        im["w3p"] = np.stack([p3[0], p3[1 + e0], p3[1 + e1]])
        im["w2t"] = np.stack([p2[0], p2[1 + e0], p2[1 + e1]])
        tok_ids.append(ids_pair)
        in_maps.append(im)

    meta = {"counts": counts, "tok_ids": tok_ids, "slot_experts": slot_experts,
            "caps": caps, "shape": hidden_states.shape}
    return nc, in_maps, meta


def _combine(results, meta):
    out = np.empty((T, H), np.float32)
    for c in range(NCORES):
        out[c * NS:(c + 1) * NS] = results[c]["ys"]
    for c in range(NCORES):
        for j in range(EPC):
            ids = meta["tok_ids"][c][j]
            out[ids] += results[c][f"ye{j}"][:len(ids)]
    return out.reshape(meta["shape"])


def kernel(hidden_states, gate_w, bias, shared_w1, shared_w3, shared_w2,
           w1, w3, w2):
    args = [np.asarray(a) for a in (hidden_states, gate_w, bias, shared_w1,
                                    shared_w3, shared_w2, w1, w3, w2)]
    nc, in_maps, meta = _prepare(*args, MM_MODE)
    global LAST_RESULTS
    try:
        res = run_bass_kernel_spmd(nc, in_maps, list(range(NCORES)))
    except Exception:
        # transient NRT device errors happen; one retry clears them
        res = run_bass_kernel_spmd(nc, in_maps, list(range(NCORES)))
    LAST_RESULTS = res
    return _combine(res.results, meta)


# revision 21
# speedup vs baseline: 1.0089x; 1.0089x over previous
"""DeepSeekV3-style MoE layer on 8 Trainium2 NeuronCores.

Strategy (expert-parallel, host-side dispatch):
  - Host computes the sigmoid gate + top-2 routing (tiny: [8192,2048]@[2048,16]),
    gathers each expert's tokens. Experts are paired largest-with-smallest and
    sharded 2-per-core; per-slot capacities C0/C1 are the max count over the
    slot's 8 experts (exact, no rounding). The shared expert is data-parallel
    (1024 tokens per core).
  - Each core runs the same Bass/Tile program: 3 SwiGLU "units"
    (shared + 2 experts), weight-stationary matmuls at N<=512 in fp16
    (full PE rate, fast weight loads) with fp32 PSUM accumulation
    (measured ~742 us/core, rel err ~6e-4; MOE_MM_MODE=f32r gives 2.9e-4
    at ~813 us if more precision is ever needed).
  - Gating scale is applied on-device during PSUM->SBUF evacuation; host
    scatter-adds expert outputs back and adds the shared output.

Layouts (host-prepared so every DMA is wide & contiguous):
  x*T   [16,128,n]        tokens transposed, h-tile major
  w1p   [3,11,128,2048]   phase-1 lhsT packs: [u][it][p=h%128][ht*128+j(=i%128)]
  w3p   same
  w2t   [3,11,128,2048]   w2 transposed: [u][it][p=i%128][h]
  gs*   [128,ct]          per-token gating scale for each local expert slot
Outputs: ys [1024,2048], ye0 [C0,2048], ye1 [C1,2048] (token-major, fp32).
"""

import os
import sys

import numpy as np

if "/opt/trn_rl_repo" not in sys.path:
    sys.path.insert(0, "/opt/trn_rl_repo")

import concourse.bass as bass
import concourse.bacc as bacc
import concourse.mybir as mybir
import concourse.tile as tile
from concourse.bass_utils import run_bass_kernel_spmd

B, S, H, I, E, TOPK = 4, 2048, 2048, 1408, 16, 2
T = B * S               # 8192 tokens
NCORES = 8
NS = T // NCORES        # shared-expert tokens per core
HT, IT = H // 128, I // 128   # 16, 11
EPC = E // NCORES       # experts per core = 2

MM_MODE = os.environ.get("MOE_MM_MODE", "fp16")   # "fp16" | "f32r" | "bf16" | "f32"

LAST_RESULTS = None     # BassKernelResults of the last run (for test harness)

_PROGRAM_CACHE = {}
_PACK_CACHE = {}


def _sigmoid(x):
    out = np.empty_like(x)
    np.negative(x, out=out)
    np.exp(out, out=out)
    out += 1.0
    np.reciprocal(out, out=out)
    return out


def _chunks(n):
    """Split n into chunks <=512, all >=256 when n allows (f32r matmul runs
    at 1/4 rate below a 256-wide moving dim)."""
    out, rem = [], n
    while rem > 0:
        if rem <= 512:
            c = rem
        elif rem >= 768:
            c = 512
        else:  # rem in (512, 768): split so both pieces are >= 256
            c = rem - 256
        out.append(c)
        rem -= c
    return out


def _build_program(caps, mode):
    """caps = (C0, C1): exact token capacity of the two local expert slots."""
    key = (caps, mode)
    if key in _PROGRAM_CACHE:
        return _PROGRAM_CACHE[key]

    if mode == "bf16":
        in_dt = mybir.dt.bfloat16
    elif mode == "fp16":
        in_dt = mybir.dt.float16
    elif mode == "f32r":
        in_dt = mybir.dt.float32r
    else:
        in_dt = mybir.dt.float32
    f32 = mybir.dt.float32
    n_units = [NS, caps[0], caps[1]]
    cts = [-(-n // 128) for n in n_units]
    CW = max(n_units)       # tile width shared by xt/g tags

    nc = bacc.Bacc("TRN2", target_bir_lowering=False, debug=False)

    xT = [nc.dram_tensor(f"x{u}T", [HT, 128, n_units[u]], in_dt,
                         kind="ExternalInput").ap() for u in range(3)]
    w1p = nc.dram_tensor("w1p", [3, IT, 128, H], in_dt, kind="ExternalInput").ap()
    w3p = nc.dram_tensor("w3p", [3, IT, 128, H], in_dt, kind="ExternalInput").ap()
    w2t = nc.dram_tensor("w2t", [3, IT, 128, H], in_dt, kind="ExternalInput").ap()
    gs = [None] + [nc.dram_tensor(f"gs{u}", [128, cts[u]], f32,
                                  kind="ExternalInput").ap() for u in (1, 2)]
    yo = [nc.dram_tensor(["ys", "ye0", "ye1"][u], [n_units[u], H], f32,
                         kind="ExternalOutput").ap() for u in range(3)]

    # DMA-queue plan (one HW queue per engine, FIFO): sync carries only the
    # activation loads, scalar only the output writes (its sigmoid compute is
    # phase-1-only), gpsimd all weight streams. Emission order = descriptor
    # order, so prefetches are hoisted ahead of the compute that needs them.
    with tile.TileContext(nc) as tc:
        with (
            tc.tile_pool(name="xt", bufs=HT) as xt_pool,
            tc.tile_pool(name="g", bufs=IT) as g_pool,
            tc.tile_pool(name="w13", bufs=6) as w13_pool,
            tc.tile_pool(name="w2", bufs=2 * IT) as w2_pool,
            tc.tile_pool(name="gsb", bufs=2) as gs_pool,
            tc.tile_pool(name="ot", bufs=4) as out_pool,
            tc.tile_pool(name="ps1", bufs=4, space="PSUM") as ps1_pool,
            tc.tile_pool(name="ps2", bufs=4, space="PSUM") as ps2_pool,
        ):
            def load_xt(u):
                n_u = n_units[u]
                xts = [xt_pool.tile([128, CW], in_dt, tag="xt",
                                    name=f"xt{u}_{ht}") for ht in range(HT)]
                if u == 0:
                    # racing the kernel start: two queues, and land the first
                    # 512 token-columns of every h-tile first so the first
                    # phase-1 chunk group can begin while the rest streams
                    w0 = min(512, n_u)
                    for ht in range(HT):
                        eng = nc.scalar if ht % 2 else nc.sync
                        eng.dma_start(out=xts[ht][:, :w0], in_=xT[u][ht][:, :w0])
                    if w0 < n_u:
                        for ht in range(HT):
                            eng = nc.scalar if ht % 2 else nc.sync
                            eng.dma_start(out=xts[ht][:, w0:n_u],
                                          in_=xT[u][ht][:, w0:])
                else:
                    for ht in range(HT):
                        nc.sync.dma_start(out=xts[ht][:, :n_u], in_=xT[u][ht])
                return xts

            xts = load_xt(0)
            for u in range(3):
                n_u = n_units[u]

                # ---- weight-stream emission (gpsimd): w13 it0/it1 first,
                # then this unit's gating vector + first w2 group, then the
                # rest of w13 (slot-waits pace the queue to compute progress)
                w13s = []
                for it in range(IT):
                    w1t = w13_pool.tile([128, H], in_dt, tag="w13",
                                        name=f"w1t{u}_{it}")
                    w3t = w13_pool.tile([128, H], in_dt, tag="w13",
                                        name=f"w3t{u}_{it}")
                    w13s.append((w1t, w3t))
                    nc.gpsimd.dma_start(out=w1t[:], in_=w1p[u, it])
                    nc.gpsimd.dma_start(out=w3t[:], in_=w3p[u, it])
                    if it == 1:
                        if u > 0:
                            gst = gs_pool.tile([128, cts[u]], f32, tag="gsb",
                                               name=f"gst{u}")
                            nc.gpsimd.dma_start(out=gst[:], in_=gs[u])
                        w2s = []
                        for w2i in range(IT):
                            w2tile = w2_pool.tile([128, 512], in_dt, tag="w2",
                                                  name=f"w2_{u}_0_{w2i}")
                            nc.gpsimd.dma_start(
                                out=w2tile[:], in_=w2t[u, w2i, :, 0:512])
                            w2s.append(w2tile)

                # ---- phase 1: G^T[i, t] = silu(W1 xT) * (W3 xT) ----
                gts = []
                for it in range(IT):
                    w1t, w3t = w13s[it]
                    gt = g_pool.tile([128, CW], in_dt, tag="g", name=f"g{u}_{it}")
                    gts.append(gt)
                    c0 = 0
                    for w in _chunks(n_u):
                        ps1 = ps1_pool.tile([128, 512], f32, tag="ps1",
                                            name=f"ps1_{u}_{it}_{c0}")
                        ps3 = ps1_pool.tile([128, 512], f32, tag="ps1",
                                            name=f"ps3_{u}_{it}_{c0}")
                        for ht in range(HT):
                            nc.tensor.matmul(
                                ps1[:, :w], w1t[:, ht * 128:(ht + 1) * 128],
                                xts[ht][:, c0:c0 + w],
                                start=(ht == 0), stop=(ht == HT - 1))
                        for ht in range(HT):
                            nc.tensor.matmul(
                                ps3[:, :w], w3t[:, ht * 128:(ht + 1) * 128],
                                xts[ht][:, c0:c0 + w],
                                start=(ht == 0), stop=(ht == HT - 1))
                        # silu(h1)*h3 = sigmoid(h1)*h1*h3 (Silu not in CoreSim)
                        gsl = gt[:, c0:c0 + w]
                        nc.scalar.activation(gsl, ps1[:, :w],
                                             mybir.ActivationFunctionType.Sigmoid)
                        nc.vector.tensor_mul(gsl, gsl, ps1[:, :w])
                        nc.vector.tensor_mul(gsl, gsl, ps3[:, :w])
                        c0 += w

                # next unit's activations stream during phase 2
                if u < 2:
                    next_xts = load_xt(u + 1)

                # ---- phase 2: Y[t, h] = G^T.T @ W2^T, +gating scale ----
                for hc in range(H // 512):
                    if hc + 1 < H // 512:   # prefetch next w2 group
                        nxt = []
                        for w2i in range(IT):
                            w2tile = w2_pool.tile([128, 512], in_dt, tag="w2",
                                                  name=f"w2_{u}_{hc + 1}_{w2i}")
                            nc.gpsimd.dma_start(
                                out=w2tile[:],
                                in_=w2t[u, w2i, :,
                                        (hc + 1) * 512:(hc + 2) * 512])
                            nxt.append(w2tile)
                    for tt in range(cts[u]):
                        m = min(128, n_u - tt * 128)
                        ps = ps2_pool.tile([128, 512], f32, tag="ps2",
                                           name=f"ps2_{u}_{hc}_{tt}")
                        for it in range(IT):
                            nc.tensor.matmul(
                                ps[:m], gts[it][:, tt * 128:tt * 128 + m],
                                w2s[it][:],
                                start=(it == 0), stop=(it == IT - 1))
                        ot = out_pool.tile([128, 512], f32, tag="ot",
                                           name=f"ot{u}_{hc}_{tt}")
                        if u == 0:
                            nc.vector.tensor_copy(ot[:m], ps[:m])
                        else:
                            nc.vector.tensor_scalar(
                                ot[:m], ps[:m], gst[:m, tt:tt + 1], None,
                                mybir.AluOpType.mult)
                        nc.scalar.dma_start(
                            out=yo[u][tt * 128:tt * 128 + m,
                                      hc * 512:(hc + 1) * 512],
                            in_=ot[:m])
                    if hc + 1 < H // 512:
                        w2s = nxt
                if u < 2:
                    xts = next_xts

    nc.compile()
    _PROGRAM_CACHE[key] = nc
    return nc


def _np_dt(mode):
    if mode == "bf16":
        import ml_dtypes
        return np.dtype(ml_dtypes.bfloat16)
    if mode == "fp16":
        return np.dtype(np.float16)
    return np.dtype(np.float32)


def _pack_w13(w, dt):
    """[I,H] -> [IT,128,H] with [it, p, ht*128+j] = w[it*128+j, ht*128+p]."""
    a = np.ascontiguousarray(
        w.reshape(IT, 128, HT, 128).transpose(0, 3, 2, 1), dtype=dt)
    return a.reshape(IT, 128, H)


def _pack_w2(w, dt):
    """[H,I] -> [IT,128,H]  (= w.T tiled along I)."""
    return np.ascontiguousarray(w.T.reshape(IT, 128, H), dtype=dt)


def _pack_all_weights(shared_w1, shared_w3, shared_w2, w1, w3, w2, mode):
    key = (id(w1), id(w2), id(w3), mode)
    if _PACK_CACHE.get("key") == key:
        return _PACK_CACHE["val"]
    dt = _np_dt(mode)
    p1 = [_pack_w13(shared_w1, dt)] + [_pack_w13(w1[e], dt) for e in range(E)]
    p3 = [_pack_w13(shared_w3, dt)] + [_pack_w13(w3[e], dt) for e in range(E)]
    p2 = [_pack_w2(shared_w2, dt)] + [_pack_w2(w2[e], dt) for e in range(E)]
    val = (p1, p3, p2)
    _PACK_CACHE["key"] = key
    _PACK_CACHE["val"] = val
    return val


def _prepare(hidden_states, gate_w, bias, shared_w1, shared_w3, shared_w2,
             w1, w3, w2, mode):
    """Host routing + per-core input maps. Returns (nc, in_maps, meta)."""
    x = np.ascontiguousarray(hidden_states.reshape(T, H), dtype=np.float32)

    scores = _sigmoid(x @ gate_w.T.astype(np.float32))
    routing = scores + bias.astype(np.float32)[None, :]
    topk = np.argsort(-routing, axis=1, kind="stable")[:, :TOPK]
    sel = np.take_along_axis(scores, topk, axis=1)
    gating = (sel / sel.sum(axis=1, keepdims=True)).astype(np.float32)

    flat_t = np.repeat(np.arange(T), TOPK)
    flat_e = topk.ravel()
    flat_g = gating.ravel()
    order = np.argsort(flat_e, kind="stable")
    flat_t, flat_g = flat_t[order], flat_g[order]
    counts = np.bincount(flat_e, minlength=E)
    offs = np.zeros(E + 1, np.int64)
    np.cumsum(counts, out=offs[1:])

    # pair largest with smallest: slot0 = rank c, slot1 = rank 15-c
    rank = np.argsort(-counts, kind="stable")
    slot_experts = [(int(rank[c]), int(rank[E - 1 - c])) for c in range(NCORES)]
    C0 = max(1, int(counts[rank[0]]))
    C1 = max(1, int(counts[rank[NCORES]]))
    caps = (C0, C1)

    nc = _build_program(caps, mode)
    dt = _np_dt(mode)

    p1, p3, p2 = _pack_all_weights(shared_w1, shared_w3, shared_w2,
                                   w1, w3, w2, mode)
    xc = x.astype(dt, copy=False)

    tok_ids = []
    in_maps = []
    for c in range(NCORES):
        im = {"x0T": np.ascontiguousarray(
            xc[c * NS:(c + 1) * NS].T).reshape(HT, 128, NS)}
        ids_pair = []
        for j, e in enumerate(slot_experts[c]):
            Cj = caps[j]
            ids = flat_t[offs[e]:offs[e + 1]]
            ids_pair.append(ids)
            n = len(ids)
            xg = np.zeros((Cj, H), dt)
            xg[:n] = xc[ids]
            im[f"x{j + 1}T"] = np.ascontiguousarray(xg.T).reshape(HT, 128, Cj)
            ct = -(-Cj // 128)
            gsc = np.zeros((ct, 128), np.float32)
            gsc.reshape(-1)[:n] = flat_g[offs[e]:offs[e + 1]]
            im[f"gs{j + 1}"] = np.ascontiguousarray(gsc.T)   # [128, ct]
        e0, e1 = slot_experts[c]
        im["w1p"] = np.stack([p1[0], p1[1 + e0], p1[1 + e1]])
        im["w3p"] = np.stack([p3[0], p3[1 + e0], p3[1 + e1]])
        im["w2t"] = np.stack([p2[0], p2[1 + e0], p2[1 + e1]])
        tok_ids.append(ids_pair)
        in_maps.append(im)

    meta = {"counts": counts, "tok_ids": tok_ids, "slot_experts": slot_experts,
            "caps": caps, "shape": hidden_states.shape}
    return nc, in_maps, meta


def _combine(results, meta):
    out = np.empty((T, H), np.float32)
    for c in range(NCORES):
        out[c * NS:(c + 1) * NS] = results[c]["ys"]
    for c in range(NCORES):
        for j in range(EPC):
            ids = meta["tok_ids"][c][j]
            out[ids] += results[c][f"ye{j}"][:len(ids)]
    return out.reshape(meta["shape"])


def kernel(hidden_states, gate_w, bias, shared_w1, shared_w3, shared_w2,
           w1, w3, w2):
    args = [np.asarray(a) for a in (hidden_states, gate_w, bias, shared_w1,
                                    shared_w3, shared_w2, w1, w3, w2)]
    nc, in_maps, meta = _prepare(*args, MM_MODE)
    global LAST_RESULTS
    try:
        res = run_bass_kernel_spmd(nc, in_maps, list(range(NCORES)))
    except Exception:
        # transient NRT device errors happen; one retry clears them
        res = run_bass_kernel_spmd(nc, in_maps, list(range(NCORES)))
    LAST_RESULTS = res
    return _combine(res.results, meta)
